# revision 35
# baseline (speedup 1.0000x reference)
"""GAT-style message passing kernel for Trainium2 (8 NeuronCores, data-parallel over batch).

Reference math (per sample, 2 layers, shared weights):
    hidden = x @ W_in + b_in                      # [N, H]
    per layer:
        xt  = hidden @ W_t + b_t
        s_j = xt @ a_j ; s_i = xt @ a_i           # xt only feeds the scores
        att = softmax_j(lrelu(s_i[i] + s_j[j]))
        hidden = att @ hidden + hidden

Restructurings used here:
 1) W_t folding: s = hidden @ (W_t a) + b_t.a  — the NxHxH transform collapses.
 2) Rank-21 factorization: hidden == U @ V with V = [W_in; b_in] constant and
    U0 = [x | 1];  per layer U <- att @ U + U  (attention commutes with V).
    All attention matmuls run on U's 21 columns; V is applied once at the end.
    The ones-column of U doubles per layer (att rows sum to 1), and its output
    row in E.T @ U equals 2^L * D — the softmax denominator comes for free.
 3) exp(lrelu(z)-C_i) = max(e^{z-C_i}, e^{0.01z-C_i}) and with C_i = s_i+maxS
    both branches are rank-1:  E[j,i] = max(p[j], p'[j]*g[i])  with
    p = e^{s_j-maxS}, p' = e^{0.01(s_j-maxS)}, g = e^{min(-0.99(s_i+maxS+c),80)}
    so the whole N^2 pass is ONE fused DVE tensor_scalar per tile, no N^2 exp.
 4) s for the next layer from the same product: s' = rD * (Y_U @ w21) + s.
"""

import numpy as np
from contextlib import ExitStack

S = 2          # samples per core
N = 2048
Din = 20
UD = Din + 1   # U columns: 20 x-features + ones
H = 128
NCH = 16       # j-chunks of 128
NB = 4         # i-blocks
FB = 512       # i-block width
NUM_LAYERS = 2
N_CORES = 8


def _build(ctx, tc, aps, ctot):
    import concourse.bass as bass
    from concourse import mybir
    from concourse.masks import make_identity

    nc = tc.nc
    f32 = mybir.dt.float32
    f16 = mybir.dt.float16
    Alu = mybir.AluOpType
    Act = mybir.ActivationFunctionType

    x_ap, w21_ap, v_ap, ident_ap, s0_ap, sel_ap, p0_ap, g0_ap, out_ap = aps

    consts = ctx.enter_context(tc.tile_pool(name="consts", bufs=1))
    utp = ctx.enter_context(tc.tile_pool(name="utp", bufs=2))        # U0T / YUT rows [UD, N]
    natp = ctx.enter_context(tc.tile_pool(name="natp", bufs=4))      # U_nat f32 [128, 16, UD]
    natp16 = ctx.enter_context(tc.tile_pool(name="natp16", bufs=4))  # U_nat fp16
    ynat = ctx.enter_context(tc.tile_pool(name="ynat", bufs=2))      # Ynat f32 [128, 16, UD]
    xin = ctx.enter_context(tc.tile_pool(name="xin", bufs=4))        # x load tiles
    gpool = ctx.enter_context(tc.tile_pool(name="gpool", bufs=3))    # gbc [128, 512]
    epool = ctx.enter_context(tc.tile_pool(name="epool", bufs=6))    # E tiles [128, 512] f16
    outp = ctx.enter_context(tc.tile_pool(name="outp", bufs=2))      # final hidden [128,16,128]
    small = ctx.enter_context(tc.tile_pool(name="small", bufs=12))
    psA = ctx.enter_context(tc.tile_pool(name="psA", bufs=1, space="PSUM"))  # ubc [128,512]
    psU = ctx.enter_context(tc.tile_pool(name="psU", bufs=4, space="PSUM"))  # YUT [UD,512]
    psT = ctx.enter_context(tc.tile_pool(name="psT", bufs=3, space="PSUM"))  # transposes

    s0_tiles, p0_tiles, g0_tiles = [], [], []
    for s in range(S):
        s0 = small.tile([128, NCH, 2], f32, tag="s0", name=f"s0_{s}")
        nc.sync.dma_start(out=s0, in_=s0_ap[s])
        s0_tiles.append(s0)
        p0 = small.tile([128, NCH, 2], f32, tag="p0", name=f"p0_{s}")
        nc.sync.dma_start(out=p0, in_=p0_ap[s])
        p0_tiles.append(p0)
        g0 = small.tile([NCH, 128], f16, tag="g16", name=f"g0_{s}")
        nc.sync.dma_start(out=g0, in_=g0_ap[s])
        g0_tiles.append(g0)
    xflats = []
    for s in range(S):
        xflat = xin.tile([128, NCH, Din], f32, name=f"xflat{s}")
        nc.sync.dma_start(out=xflat, in_=x_ap[s].rearrange("(p c) d -> p c d", c=NCH))
        xflats.append(xflat)
    ident = consts.tile([128, 128], f32)
    nc.sync.dma_start(out=ident, in_=ident_ap)
    ones_r = consts.tile([1, 128], f32)
    nc.vector.memset(ones_r, 1.0)
    w21_sb = consts.tile([UD, 2], f32)
    nc.sync.dma_start(out=w21_sb, in_=w21_ap)
    v_sb = consts.tile([UD, H], f32)
    nc.sync.dma_start(out=v_sb, in_=v_ap)
    selmat16 = consts.tile([NCH, NCH, 128], f16)
    nc.sync.dma_start(out=selmat16, in_=sel_ap)

    def ts(out, in0, s1, s2, op0, op1=None):
        if op1 is None:
            nc.vector.tensor_scalar(out, in0, s1, None, op0)
        else:
            nc.vector.tensor_scalar(out, in0, s1, s2, op0, op1)

    # ------------- input stage: x -> U0 (natural + T), initial scores -------
    u_nat = [None, None]
    u_nat16 = [None, None]
    s_part = [None, None]   # biasless scores [128, 16, 2]
    for s in range(S):
        # node n lives at (partition p, chunk c) with n = 16 p + c —
        # a fixed relabeling the attention sum is invariant to.
        # Initial biasless scores s0 = [x|1] @ w21 are input-only: folded on host.
        xflat = xflats[s]
        un = natp.tile([128, NCH, UD], f32, tag="unat")
        nc.vector.memset(un[:, :, Din:UD], 1.0)
        nc.vector.tensor_copy(un[:, :, 0:Din], xflat)
        un16 = natp16.tile([128, NCH, UD], f16, tag="unat16")
        nc.scalar.copy(un16, un)
        u_nat[s], u_nat16[s], s_part[s] = un, un16, s0_tiles[s]

    # ------------- layers ---------------------------------------------------
    # Emission schedule (PE is in-order per engine): each sample's finalize +
    # next-layer prep is emitted right after its own sweep, so it executes
    # while the OTHER sample's sweep occupies the PE.
    prep = {}
    yuts = {}
    finals = {}

    def emit_prep(s, L):
        un, un16, s0 = u_nat[s], u_nat16[s], s_part[s]
        if L == 0:
            # p, p', g for layer 0 depend only on inputs: host-folded
            p0 = p0_tiles[s]
            gbc = gpool.tile([128, N], f16, tag="gbc")
            for b in range(NB):
                ubc = psA.tile([128, FB], f32, tag="ubc")
                for k in range(4):
                    c = 4 * b + k
                    nc.tensor.matmul(ubc[:, k * 128:(k + 1) * 128],
                                     lhsT=selmat16[:, c, :], rhs=g0_tiles[s],
                                     start=True, stop=True)
                nc.scalar.copy(gbc[:, b * FB:(b + 1) * FB], ubc)
            prep[s] = (p0[:, :, 0], p0[:, :, 1], gbc)
            return
        m1 = small.tile([128, 1], f32, tag="m1")
        nc.vector.tensor_reduce(m1, s0[:, :, 0], axis=mybir.AxisListType.X, op=Alu.max)
        psm = psT.tile([1, 128], f32, tag="tp")
        nc.tensor.matmul(psm, lhsT=m1, rhs=ident, start=True, stop=True)
        m1r = small.tile([1, 128], f32, tag="m1r")
        nc.scalar.copy(m1r, psm)
        mx = small.tile([1, 1], f32, tag="mx")
        nc.vector.tensor_reduce(mx, m1r, axis=mybir.AxisListType.X, op=Alu.max)
        psmb = psT.tile([128, 1], f32, tag="tp")
        nc.tensor.matmul(psmb, lhsT=ones_r, rhs=mx, start=True, stop=True)
        maxbc = small.tile([128, 1], f32, tag="maxbc")
        nc.scalar.copy(maxbc, psmb)
        negmax = small.tile([128, 1], f32, tag="negmax")
        ts(negmax, maxbc, -1.0, None, Alu.mult)
        negmax001 = small.tile([128, 1], f32, tag="negmax001")
        ts(negmax001, maxbc, -0.01, None, Alu.mult)
        p_sb = small.tile([128, NCH], f32, tag="p_sb")
        nc.scalar.activation(p_sb, s0[:, :, 0], Act.Exp, bias=negmax[:, 0:1], scale=1.0)
        pp_sb = small.tile([128, NCH], f32, tag="pp_sb")
        nc.scalar.activation(pp_sb, s0[:, :, 0], Act.Exp, bias=negmax001[:, 0:1], scale=0.01)
        u1 = small.tile([128, NCH], f32, tag="u1")
        ts(u1, s0[:, :, 1], maxbc[:, 0:1], float(ctot), Alu.add, Alu.add)
        u_sb = small.tile([128, NCH], f32, tag="u_sb")
        ts(u_sb, u1, -0.99, 10.5, Alu.mult, Alu.min)
        psuT = psT.tile([NCH, 128], f32, tag="tp")
        nc.tensor.transpose(psuT, u_sb, ident)
        g16 = small.tile([NCH, 128], f16, tag="g16")
        nc.scalar.activation(g16, psuT, Act.Exp)
        gbc = gpool.tile([128, N], f16, tag="gbc")
        for b in range(NB):
            ubc = psA.tile([128, FB], f32, tag="ubc")
            for k in range(4):
                c = 4 * b + k
                nc.tensor.matmul(ubc[:, k * 128:(k + 1) * 128],
                                 lhsT=selmat16[:, c, :], rhs=g16,
                                 start=True, stop=True)
            nc.scalar.copy(gbc[:, b * FB:(b + 1) * FB], ubc)
        prep[s] = (p_sb, pp_sb, gbc)

    def emit_sweep(s):
        p_sb, pp_sb, gbc = prep[s]
        un16 = u_nat16[s]
        yut_sb = utp.tile([UD, N], f32, tag="yut", name=f"yut{s}")
        W2 = 2 * FB
        for bb in range(NB // 2):
            yps0 = psU.tile([UD, FB], f32, tag="yps", name="yps0")
            yps1 = psU.tile([UD, FB], f32, tag="yps", name="yps1")
            etiles = []
            for c in range(NCH):
                e_t = epool.tile([128, W2], f16, tag="e", name=f"e{c}")
                ts(e_t, gbc[:, bb * W2:(bb + 1) * W2], pp_sb[:, c:c + 1],
                   p_sb[:, c:c + 1], Alu.mult, Alu.max)
                etiles.append(e_t)
            for c in range(NCH):
                nc.tensor.matmul(yps0, lhsT=un16[:, c, :], rhs=etiles[c][:, 0:FB],
                                 start=(c == 0), stop=(c == NCH - 1))
                nc.tensor.matmul(yps1, lhsT=un16[:, c, :], rhs=etiles[c][:, FB:W2],
                                 start=(c == 0), stop=(c == NCH - 1))
            nc.scalar.copy(yut_sb[:, bb * W2:bb * W2 + FB], yps0)
            nc.scalar.copy(yut_sb[:, bb * W2 + FB:(bb + 1) * W2], yps1)
        yuts[s] = yut_sb

    def emit_fin(s, L, last):
        un, s0, yut_sb = u_nat[s], s_part[s], yuts[s]
        yn = ynat.tile([128, NCH, UD], f32, tag="ynat")
        for c in range(NCH):
            pst = psT.tile([128, UD], f32, tag="tp")
            nc.tensor.transpose(pst, yut_sb[:, c * 128:(c + 1) * 128],
                                ident[0:UD, 0:UD])
            nc.scalar.copy(yn[:, c, :], pst)
        dsc = small.tile([128, NCH], f32, tag="dsc")
        ts(dsc, yn[:, :, Din], float(2.0 ** (-L)), None, Alu.mult)
        rd = small.tile([128, NCH], f32, tag="rd")
        nc.vector.reciprocal(rd, dsc)
        new_un = natp.tile([128, NCH, UD], f32, tag="unat")
        for c in range(NCH):
            nc.vector.scalar_tensor_tensor(new_un[:, c, :], yn[:, c, :],
                                           rd[:, c:c + 1], un[:, c, :],
                                           Alu.mult, Alu.add)
        if not last:
            new_un16 = natp16.tile([128, NCH, UD], f16, tag="unat16")
            nc.scalar.copy(new_un16, new_un)
            psq = psT.tile([128, 32], f32, tag="tp")
            for c in range(NCH):
                nc.tensor.matmul(psq[:, 2 * c:2 * c + 2],
                                 lhsT=yut_sb[:, c * 128:(c + 1) * 128],
                                 rhs=w21_sb, start=True, stop=True)
            qp = small.tile([128, NCH, 2], f32, tag="qp")
            nc.scalar.copy(qp, psq.rearrange("p (c z) -> p c z", z=2))
            new_s0 = small.tile([128, NCH, 2], f32, tag="s0")
            for c in range(NCH):
                nc.vector.scalar_tensor_tensor(new_s0[:, c, :], qp[:, c, :],
                                               rd[:, c:c + 1], s0[:, c, :],
                                               Alu.mult, Alu.add)
            u_nat[s], u_nat16[s], s_part[s] = new_un, new_un16, new_s0
        else:
            finals[s] = new_un

    # L0: fin+prep(L1) of each sample emitted right after its own sweep
    emit_prep(0, 0)
    emit_prep(1, 0)
    emit_sweep(0)
    emit_fin(0, 0, last=False)
    emit_prep(0, 1)
    emit_sweep(1)
    emit_fin(1, 0, last=False)
    emit_prep(1, 1)
    # L1
    emit_sweep(0)
    emit_fin(0, 1, last=True)
    emit_sweep(1)
    emit_fin(1, 1, last=True)

    # final tail: hidden = U' @ V, samples interleaved, grouped output DMA
    houts = {s: outp.tile([128, NCH, H], f32, tag="hout", name=f"hout{s}")
             for s in range(S)}
    for c in range(NCH):
        for s in range(S):
            psut = psU.tile([UD, 128], f32, tag="yps")
            nc.tensor.transpose(psut, finals[s][:, c, :], ident)
            u2t_c = small.tile([UD, 128], f32, tag="u2t")
            nc.scalar.copy(u2t_c, psut)
            psh = psT.tile([128, H], f32, tag="tp")
            nc.tensor.matmul(psh, lhsT=u2t_c, rhs=v_sb, start=True, stop=True)
            nc.vector.tensor_copy(houts[s][:, c, :], psh)
        if c % 4 == 3:
            for s in range(S):
                nc.sync.dma_start(
                    out=out_ap[s].rearrange("(p c) h -> p c h", c=NCH)[:, c - 3:c + 1, :],
                    in_=houts[s][:, c - 3:c + 1, :])

def _host_prep(inputs):
    x = np.ascontiguousarray(np.asarray(inputs["x"], dtype=np.float32))
    W_in = np.asarray(inputs["W_in"], dtype=np.float32)
    b_in = np.asarray(inputs["b_in"], dtype=np.float32)
    W_t = np.asarray(inputs["W_t"], dtype=np.float32)
    b_t = np.asarray(inputs["b_t"], dtype=np.float32)
    a = np.asarray(inputs["a"], dtype=np.float32)
    a_j, a_i = a[:H, 0], a[H:, 0]
    wj = (W_t @ a_j).astype(np.float32)
    wi = (W_t @ a_i).astype(np.float32)
    V = np.ascontiguousarray(np.concatenate([W_in, b_in[None, :]], axis=0))  # [21, 128]
    w21 = np.ascontiguousarray(np.stack([V @ wj, V @ wi], axis=1))           # [21, 2]
    ctot = float(np.float32(b_t @ a_j) + np.float32(b_t @ a_i))
    B = x.shape[0]
    U0 = np.concatenate([x, np.ones((B, N, 1), np.float32)], axis=2)
    s0 = (U0 @ w21).astype(np.float32).reshape(B, 128, NCH, 2)  # n = 16p + c
    s0 = np.ascontiguousarray(s0)
    sel = np.zeros((NCH, NCH, 128), np.float16)
    for c in range(NCH):
        sel[c, c, :] = 1.0
    s0j, s0i = s0[..., 0], s0[..., 1]
    mx = s0j.max(axis=(1, 2), keepdims=True)
    p0 = np.stack([np.exp(s0j - mx), np.exp(0.01 * (s0j - mx))], axis=3).astype(np.float32)
    u0 = np.minimum(-0.99 * (s0i + mx + np.float32(ctot)), 10.5).astype(np.float32)
    g0 = np.ascontiguousarray(np.exp(u0).astype(np.float16).transpose(0, 2, 1))
    return x, w21, V, ctot, s0, sel, p0, g0


def build_program(ctot):
    import concourse.tile as tile
    from concourse import mybir
    from concourse.bacc import Bacc

    f32 = mybir.dt.float32
    nc = Bacc("TRN2", target_bir_lowering=False, debug=False)
    x_t = nc.dram_tensor("x", [S, N, Din], f32, kind="ExternalInput")
    w21_t = nc.dram_tensor("w21", [UD, 2], f32, kind="ExternalInput")
    v_t = nc.dram_tensor("v", [UD, H], f32, kind="ExternalInput")
    ident_t = nc.dram_tensor("ident", [128, 128], f32, kind="ExternalInput")
    s0_t = nc.dram_tensor("s0in", [S, 128, NCH, 2], f32, kind="ExternalInput")
    sel_t = nc.dram_tensor("sel16", [NCH, NCH, 128], mybir.dt.float16, kind="ExternalInput")
    p0_t = nc.dram_tensor("p0in", [S, 128, NCH, 2], f32, kind="ExternalInput")
    g0_t = nc.dram_tensor("g0in", [S, NCH, 128], mybir.dt.float16, kind="ExternalInput")
    out_t = nc.dram_tensor("out", [S, N, H], f32, kind="ExternalOutput")
    aps = (x_t.ap(), w21_t.ap(), v_t.ap(), ident_t.ap(), s0_t.ap(), sel_t.ap(), p0_t.ap(), g0_t.ap(), out_t.ap())
    with tile.TileContext(nc) as tc, ExitStack() as ctx:
        _build(ctx, tc, aps, ctot)
    nc.compile()
    return nc


def kernel(**inputs) -> np.ndarray:
    from concourse.bass_utils import run_bass_kernel_spmd

    x, w21, V, ctot, s0, sel, p0, g0 = _host_prep(inputs)
    B = x.shape[0]
    nc = build_program(ctot)
    in_maps = []
    for i in range(N_CORES):
        in_maps.append({
            "x": np.ascontiguousarray(x[i * S:(i + 1) * S]),
            "w21": w21,
            "v": V,
            "ident": np.eye(128, dtype=np.float32),
            "s0in": np.ascontiguousarray(s0[i * S:(i + 1) * S]),
            "sel16": sel,
            "p0in": np.ascontiguousarray(p0[i * S:(i + 1) * S]),
            "g0in": np.ascontiguousarray(g0[i * S:(i + 1) * S]),
        })
    res = run_bass_kernel_spmd(nc, in_maps, list(range(N_CORES)))
    out = np.concatenate([res.results[i]["out"] for i in range(N_CORES)], axis=0)
    assert out.shape == (B, N, H)
    return out


# revision 36
# speedup vs baseline: 1.0564x; 1.0564x over previous
"""GAT-style message passing kernel for Trainium2 (8 NeuronCores, data-parallel over batch).

Reference math (per sample, 2 layers, shared weights):
    hidden = x @ W_in + b_in                      # [N, H]
    per layer:
        xt  = hidden @ W_t + b_t
        s_j = xt @ a_j ; s_i = xt @ a_i           # xt only feeds the scores
        att = softmax_j(lrelu(s_i[i] + s_j[j]))
        hidden = att @ hidden + hidden

Restructurings used here:
 1) W_t folding: s = hidden @ (W_t a) + b_t.a  — the NxHxH transform collapses.
 2) Rank-21 factorization: hidden == U @ V with V = [W_in; b_in] constant and
    U0 = [x | 1];  per layer U <- att @ U + U  (attention commutes with V).
    All attention matmuls run on U's 21 columns; V is applied once at the end.
    The ones-column of U doubles per layer (att rows sum to 1), and its output
    row in E.T @ U equals 2^L * D — the softmax denominator comes for free.
 3) exp(lrelu(z)-C_i) = max(e^{z-C_i}, e^{0.01z-C_i}) and with C_i = s_i+maxS
    both branches are rank-1:  E[j,i] = max(p[j], p'[j]*g[i])  with
    p = e^{s_j-maxS}, p' = e^{0.01(s_j-maxS)}, g = e^{min(-0.99(s_i+maxS+c),80)}
    so the whole N^2 pass is ONE fused DVE tensor_scalar per tile, no N^2 exp.
 4) s for the next layer from the same product: s' = rD * (Y_U @ w21) + s.
"""

import numpy as np
from contextlib import ExitStack

S = 2          # samples per core
N = 2048
Din = 20
UD = Din + 1   # U columns: 20 x-features + ones
H = 128
NCH = 16       # j-chunks of 128
NB = 4         # i-blocks
FB = 512       # i-block width
NUM_LAYERS = 2
N_CORES = 8


def _build(ctx, tc, aps, ctot):
    import concourse.bass as bass
    from concourse import mybir
    from concourse.masks import make_identity

    nc = tc.nc
    f32 = mybir.dt.float32
    f16 = mybir.dt.float16
    Alu = mybir.AluOpType
    Act = mybir.ActivationFunctionType

    x_ap, w21_ap, v_ap, ident_ap, s0_ap, sel_ap, p0_ap, g0_ap, out_ap = aps

    consts = ctx.enter_context(tc.tile_pool(name="consts", bufs=1))
    utp = ctx.enter_context(tc.tile_pool(name="utp", bufs=2))        # U0T / YUT rows [UD, N]
    natp = ctx.enter_context(tc.tile_pool(name="natp", bufs=4))      # U_nat f32 [128, 16, UD]
    natp16 = ctx.enter_context(tc.tile_pool(name="natp16", bufs=4))  # U_nat fp16
    ynat = ctx.enter_context(tc.tile_pool(name="ynat", bufs=2))      # Ynat f32 [128, 16, UD]
    xin = ctx.enter_context(tc.tile_pool(name="xin", bufs=4))        # x load tiles
    gpool = ctx.enter_context(tc.tile_pool(name="gpool", bufs=3))    # gbc [128, 512]
    epool = ctx.enter_context(tc.tile_pool(name="epool", bufs=6))    # E tiles [128, 512] f16
    outp = ctx.enter_context(tc.tile_pool(name="outp", bufs=2))      # final hidden [128,16,128]
    small = ctx.enter_context(tc.tile_pool(name="small", bufs=12))
    psA = ctx.enter_context(tc.tile_pool(name="psA", bufs=1, space="PSUM"))  # ubc [128,512]
    psU = ctx.enter_context(tc.tile_pool(name="psU", bufs=4, space="PSUM"))  # YUT [UD,512]
    psT = ctx.enter_context(tc.tile_pool(name="psT", bufs=3, space="PSUM"))  # transposes

    # DMA queue order = first-consumption order: the first PE ops are the
    # L0 selector matmuls (need selmat16 + g0), then the sweep needs un16
    # (xflat) and the E scalars (p0).
    selmat16 = consts.tile([NCH, NCH, 128], f16)
    nc.sync.dma_start(out=selmat16, in_=sel_ap)
    s0_tiles, p0_tiles, g0_tiles, xflats = [], [], [], []
    for s in range(S):
        g0 = small.tile([NCH, 128], f16, tag="g16", name=f"g0_{s}")
        nc.sync.dma_start(out=g0, in_=g0_ap[s])
        g0_tiles.append(g0)
    for s in range(S):
        xflat = xin.tile([128, NCH, Din], f32, name=f"xflat{s}")
        nc.sync.dma_start(out=xflat, in_=x_ap[s].rearrange("(p c) d -> p c d", c=NCH))
        xflats.append(xflat)
        p0 = small.tile([128, NCH, 2], f32, tag="p0", name=f"p0_{s}")
        nc.sync.dma_start(out=p0, in_=p0_ap[s])
        p0_tiles.append(p0)
    for s in range(S):
        s0 = small.tile([128, NCH, 2], f32, tag="s0", name=f"s0_{s}")
        nc.sync.dma_start(out=s0, in_=s0_ap[s])
        s0_tiles.append(s0)
    ident = consts.tile([128, 128], f32)
    nc.sync.dma_start(out=ident, in_=ident_ap)
    ones_r = consts.tile([1, 128], f32)
    nc.vector.memset(ones_r, 1.0)
    w21_sb = consts.tile([UD, 2], f32)
    nc.sync.dma_start(out=w21_sb, in_=w21_ap)
    v_sb = consts.tile([UD, H], f32)
    nc.sync.dma_start(out=v_sb, in_=v_ap)

    def ts(out, in0, s1, s2, op0, op1=None):
        if op1 is None:
            nc.vector.tensor_scalar(out, in0, s1, None, op0)
        else:
            nc.vector.tensor_scalar(out, in0, s1, s2, op0, op1)

    # ------------- input stage: x -> U0 (natural + T), initial scores -------
    u_nat = [None, None]
    u_nat16 = [None, None]
    s_part = [None, None]   # biasless scores [128, 16, 2]
    for s in range(S):
        # node n lives at (partition p, chunk c) with n = 16 p + c —
        # a fixed relabeling the attention sum is invariant to.
        # Initial biasless scores s0 = [x|1] @ w21 are input-only: folded on host.
        xflat = xflats[s]
        un = natp.tile([128, NCH, UD], f32, tag="unat")
        nc.vector.memset(un[:, :, Din:UD], 1.0)
        nc.vector.tensor_copy(un[:, :, 0:Din], xflat)
        un16 = natp16.tile([128, NCH, UD], f16, tag="unat16")
        nc.scalar.copy(un16, un)
        u_nat[s], u_nat16[s], s_part[s] = un, un16, s0_tiles[s]

    # ------------- layers ---------------------------------------------------
    # Emission schedule (PE is in-order per engine): each sample's finalize +
    # next-layer prep is emitted right after its own sweep, so it executes
    # while the OTHER sample's sweep occupies the PE.
    prep = {}
    yuts = {}
    finals = {}

    def emit_prep(s, L):
        un, un16, s0 = u_nat[s], u_nat16[s], s_part[s]
        if L == 0:
            # p, p', g for layer 0 depend only on inputs: host-folded
            p0 = p0_tiles[s]
            gbc = gpool.tile([128, N], f16, tag="gbc")
            for b in range(NB):
                ubc = psA.tile([128, FB], f32, tag="ubc")
                for k in range(4):
                    c = 4 * b + k
                    nc.tensor.matmul(ubc[:, k * 128:(k + 1) * 128],
                                     lhsT=selmat16[:, c, :], rhs=g0_tiles[s],
                                     start=True, stop=True)
                nc.scalar.copy(gbc[:, b * FB:(b + 1) * FB], ubc)
            prep[s] = (p0[:, :, 0], p0[:, :, 1], gbc)
            return
        m1 = small.tile([128, 1], f32, tag="m1")
        nc.vector.tensor_reduce(m1, s0[:, :, 0], axis=mybir.AxisListType.X, op=Alu.max)
        psm = psT.tile([1, 128], f32, tag="tp")
        nc.tensor.matmul(psm, lhsT=m1, rhs=ident, start=True, stop=True)
        m1r = small.tile([1, 128], f32, tag="m1r")
        nc.scalar.copy(m1r, psm)
        mx = small.tile([1, 1], f32, tag="mx")
        nc.vector.tensor_reduce(mx, m1r, axis=mybir.AxisListType.X, op=Alu.max)
        psmb = psT.tile([128, 1], f32, tag="tp")
        nc.tensor.matmul(psmb, lhsT=ones_r, rhs=mx, start=True, stop=True)
        maxbc = small.tile([128, 1], f32, tag="maxbc")
        nc.scalar.copy(maxbc, psmb)
        negmax = small.tile([128, 1], f32, tag="negmax")
        ts(negmax, maxbc, -1.0, None, Alu.mult)
        negmax001 = small.tile([128, 1], f32, tag="negmax001")
        ts(negmax001, maxbc, -0.01, None, Alu.mult)
        p_sb = small.tile([128, NCH], f32, tag="p_sb")
        nc.scalar.activation(p_sb, s0[:, :, 0], Act.Exp, bias=negmax[:, 0:1], scale=1.0)
        pp_sb = small.tile([128, NCH], f32, tag="pp_sb")
        nc.scalar.activation(pp_sb, s0[:, :, 0], Act.Exp, bias=negmax001[:, 0:1], scale=0.01)
        u1 = small.tile([128, NCH], f32, tag="u1")
        ts(u1, s0[:, :, 1], maxbc[:, 0:1], float(ctot), Alu.add, Alu.add)
        u_sb = small.tile([128, NCH], f32, tag="u_sb")
        ts(u_sb, u1, -0.99, 10.5, Alu.mult, Alu.min)
        psuT = psT.tile([NCH, 128], f32, tag="tp")
        nc.tensor.transpose(psuT, u_sb, ident)
        g16 = small.tile([NCH, 128], f16, tag="g16")
        nc.scalar.activation(g16, psuT, Act.Exp)
        gbc = gpool.tile([128, N], f16, tag="gbc")
        for b in range(NB):
            ubc = psA.tile([128, FB], f32, tag="ubc")
            for k in range(4):
                c = 4 * b + k
                nc.tensor.matmul(ubc[:, k * 128:(k + 1) * 128],
                                 lhsT=selmat16[:, c, :], rhs=g16,
                                 start=True, stop=True)
            nc.scalar.copy(gbc[:, b * FB:(b + 1) * FB], ubc)
        prep[s] = (p_sb, pp_sb, gbc)

    def emit_sweep(s):
        p_sb, pp_sb, gbc = prep[s]
        un16 = u_nat16[s]
        yut_sb = utp.tile([UD, N], f32, tag="yut", name=f"yut{s}")
        W2 = 2 * FB
        for bb in range(NB // 2):
            yps0 = psU.tile([UD, FB], f32, tag="yps", name="yps0")
            yps1 = psU.tile([UD, FB], f32, tag="yps", name="yps1")
            etiles = []
            for c in range(NCH):
                e_t = epool.tile([128, W2], f16, tag="e", name=f"e{c}")
                ts(e_t, gbc[:, bb * W2:(bb + 1) * W2], pp_sb[:, c:c + 1],
                   p_sb[:, c:c + 1], Alu.mult, Alu.max)
                etiles.append(e_t)
            for c in range(NCH):
                nc.tensor.matmul(yps0, lhsT=un16[:, c, :], rhs=etiles[c][:, 0:FB],
                                 start=(c == 0), stop=(c == NCH - 1))
                nc.tensor.matmul(yps1, lhsT=un16[:, c, :], rhs=etiles[c][:, FB:W2],
                                 start=(c == 0), stop=(c == NCH - 1))
            nc.scalar.copy(yut_sb[:, bb * W2:bb * W2 + FB], yps0)
            nc.scalar.copy(yut_sb[:, bb * W2 + FB:(bb + 1) * W2], yps1)
        yuts[s] = yut_sb

    def emit_fin(s, L, last):
        un, s0, yut_sb = u_nat[s], s_part[s], yuts[s]
        yn = ynat.tile([128, NCH, UD], f32, tag="ynat")
        for c in range(NCH):
            pst = psT.tile([128, UD], f32, tag="tp")
            nc.tensor.transpose(pst, yut_sb[:, c * 128:(c + 1) * 128],
                                ident[0:UD, 0:UD])
            nc.scalar.copy(yn[:, c, :], pst)
        dsc = small.tile([128, NCH], f32, tag="dsc")
        ts(dsc, yn[:, :, Din], float(2.0 ** (-L)), None, Alu.mult)
        rd = small.tile([128, NCH], f32, tag="rd")
        nc.vector.reciprocal(rd, dsc)
        new_un = natp.tile([128, NCH, UD], f32, tag="unat")
        for c in range(NCH):
            nc.vector.scalar_tensor_tensor(new_un[:, c, :], yn[:, c, :],
                                           rd[:, c:c + 1], un[:, c, :],
                                           Alu.mult, Alu.add)
        if not last:
            new_un16 = natp16.tile([128, NCH, UD], f16, tag="unat16")
            nc.scalar.copy(new_un16, new_un)
            psq = psT.tile([128, 32], f32, tag="tp")
            for c in range(NCH):
                nc.tensor.matmul(psq[:, 2 * c:2 * c + 2],
                                 lhsT=yut_sb[:, c * 128:(c + 1) * 128],
                                 rhs=w21_sb, start=True, stop=True)
            qp = small.tile([128, NCH, 2], f32, tag="qp")
            nc.scalar.copy(qp, psq.rearrange("p (c z) -> p c z", z=2))
            new_s0 = small.tile([128, NCH, 2], f32, tag="s0")
            for c in range(NCH):
                nc.vector.scalar_tensor_tensor(new_s0[:, c, :], qp[:, c, :],
                                               rd[:, c:c + 1], s0[:, c, :],
                                               Alu.mult, Alu.add)
            u_nat[s], u_nat16[s], s_part[s] = new_un, new_un16, new_s0
        else:
            finals[s] = new_un

    # L0: fin+prep(L1) of each sample emitted right after its own sweep
    emit_prep(0, 0)
    emit_prep(1, 0)
    emit_sweep(0)
    emit_fin(0, 0, last=False)
    emit_prep(0, 1)
    emit_sweep(1)
    emit_fin(1, 0, last=False)
    emit_prep(1, 1)
    # L1
    emit_sweep(0)
    emit_fin(0, 1, last=True)
    emit_sweep(1)
    emit_fin(1, 1, last=True)

    # final tail: hidden = U' @ V, samples interleaved, grouped output DMA
    houts = {s: outp.tile([128, NCH, H], f32, tag="hout", name=f"hout{s}")
             for s in range(S)}
    for c in range(NCH):
        for s in range(S):
            psut = psU.tile([UD, 128], f32, tag="yps")
            nc.tensor.transpose(psut, finals[s][:, c, :], ident)
            u2t_c = small.tile([UD, 128], f32, tag="u2t")
            nc.scalar.copy(u2t_c, psut)
            psh = psT.tile([128, H], f32, tag="tp")
            nc.tensor.matmul(psh, lhsT=u2t_c, rhs=v_sb, start=True, stop=True)
            nc.vector.tensor_copy(houts[s][:, c, :], psh)
        if c % 4 == 3:
            for s in range(S):
                nc.sync.dma_start(
                    out=out_ap[s].rearrange("(p c) h -> p c h", c=NCH)[:, c - 3:c + 1, :],
                    in_=houts[s][:, c - 3:c + 1, :])

def _host_prep(inputs):
    x = np.ascontiguousarray(np.asarray(inputs["x"], dtype=np.float32))
    W_in = np.asarray(inputs["W_in"], dtype=np.float32)
    b_in = np.asarray(inputs["b_in"], dtype=np.float32)
    W_t = np.asarray(inputs["W_t"], dtype=np.float32)
    b_t = np.asarray(inputs["b_t"], dtype=np.float32)
    a = np.asarray(inputs["a"], dtype=np.float32)
    a_j, a_i = a[:H, 0], a[H:, 0]
    wj = (W_t @ a_j).astype(np.float32)
    wi = (W_t @ a_i).astype(np.float32)
    V = np.ascontiguousarray(np.concatenate([W_in, b_in[None, :]], axis=0))  # [21, 128]
    w21 = np.ascontiguousarray(np.stack([V @ wj, V @ wi], axis=1))           # [21, 2]
    ctot = float(np.float32(b_t @ a_j) + np.float32(b_t @ a_i))
    B = x.shape[0]
    U0 = np.concatenate([x, np.ones((B, N, 1), np.float32)], axis=2)
    s0 = (U0 @ w21).astype(np.float32).reshape(B, 128, NCH, 2)  # n = 16p + c
    s0 = np.ascontiguousarray(s0)
    sel = np.zeros((NCH, NCH, 128), np.float16)
    for c in range(NCH):
        sel[c, c, :] = 1.0
    s0j, s0i = s0[..., 0], s0[..., 1]
    mx = s0j.max(axis=(1, 2), keepdims=True)
    p0 = np.stack([np.exp(s0j - mx), np.exp(0.01 * (s0j - mx))], axis=3).astype(np.float32)
    u0 = np.minimum(-0.99 * (s0i + mx + np.float32(ctot)), 10.5).astype(np.float32)
    g0 = np.ascontiguousarray(np.exp(u0).astype(np.float16).transpose(0, 2, 1))
    return x, w21, V, ctot, s0, sel, p0, g0


def build_program(ctot):
    import concourse.tile as tile
    from concourse import mybir
    from concourse.bacc import Bacc

    f32 = mybir.dt.float32
    nc = Bacc("TRN2", target_bir_lowering=False, debug=False)
    x_t = nc.dram_tensor("x", [S, N, Din], f32, kind="ExternalInput")
    w21_t = nc.dram_tensor("w21", [UD, 2], f32, kind="ExternalInput")
    v_t = nc.dram_tensor("v", [UD, H], f32, kind="ExternalInput")
    ident_t = nc.dram_tensor("ident", [128, 128], f32, kind="ExternalInput")
    s0_t = nc.dram_tensor("s0in", [S, 128, NCH, 2], f32, kind="ExternalInput")
    sel_t = nc.dram_tensor("sel16", [NCH, NCH, 128], mybir.dt.float16, kind="ExternalInput")
    p0_t = nc.dram_tensor("p0in", [S, 128, NCH, 2], f32, kind="ExternalInput")
    g0_t = nc.dram_tensor("g0in", [S, NCH, 128], mybir.dt.float16, kind="ExternalInput")
    out_t = nc.dram_tensor("out", [S, N, H], f32, kind="ExternalOutput")
    aps = (x_t.ap(), w21_t.ap(), v_t.ap(), ident_t.ap(), s0_t.ap(), sel_t.ap(), p0_t.ap(), g0_t.ap(), out_t.ap())
    with tile.TileContext(nc) as tc, ExitStack() as ctx:
        _build(ctx, tc, aps, ctot)
    nc.compile()
    return nc


def kernel(**inputs) -> np.ndarray:
    from concourse.bass_utils import run_bass_kernel_spmd

    x, w21, V, ctot, s0, sel, p0, g0 = _host_prep(inputs)
    B = x.shape[0]
    nc = build_program(ctot)
    in_maps = []
    for i in range(N_CORES):
        in_maps.append({
            "x": np.ascontiguousarray(x[i * S:(i + 1) * S]),
            "w21": w21,
            "v": V,
            "ident": np.eye(128, dtype=np.float32),
            "s0in": np.ascontiguousarray(s0[i * S:(i + 1) * S]),
            "sel16": sel,
            "p0in": np.ascontiguousarray(p0[i * S:(i + 1) * S]),
            "g0in": np.ascontiguousarray(g0[i * S:(i + 1) * S]),
        })
    res = run_bass_kernel_spmd(nc, in_maps, list(range(N_CORES)))
    out = np.concatenate([res.results[i]["out"] for i in range(N_CORES)], axis=0)
    assert out.shape == (B, N, H)
    return out


# revision 37
# speedup vs baseline: 1.0713x; 1.0140x over previous
"""GAT-style message passing kernel for Trainium2 (8 NeuronCores, data-parallel over batch).

Reference math (per sample, 2 layers, shared weights):
    hidden = x @ W_in + b_in                      # [N, H]
    per layer:
        xt  = hidden @ W_t + b_t
        s_j = xt @ a_j ; s_i = xt @ a_i           # xt only feeds the scores
        att = softmax_j(lrelu(s_i[i] + s_j[j]))
        hidden = att @ hidden + hidden

Restructurings used here:
 1) W_t folding: s = hidden @ (W_t a) + b_t.a  — the NxHxH transform collapses.
 2) Rank-21 factorization: hidden == U @ V with V = [W_in; b_in] constant and
    U0 = [x | 1];  per layer U <- att @ U + U  (attention commutes with V).
    All attention matmuls run on U's 21 columns; V is applied once at the end.
    The ones-column of U doubles per layer (att rows sum to 1), and its output
    row in E.T @ U equals 2^L * D — the softmax denominator comes for free.
 3) exp(lrelu(z)-C_i) = max(e^{z-C_i}, e^{0.01z-C_i}) and with C_i = s_i+maxS
    both branches are rank-1:  E[j,i] = max(p[j], p'[j]*g[i])  with
    p = e^{s_j-maxS}, p' = e^{0.01(s_j-maxS)}, g = e^{min(-0.99(s_i+maxS+c),80)}
    so the whole N^2 pass is ONE fused DVE tensor_scalar per tile, no N^2 exp.
 4) s for the next layer from the same product: s' = rD * (Y_U @ w21) + s.
"""

import numpy as np
from contextlib import ExitStack

S = 2          # samples per core
N = 2048
Din = 20
UD = Din + 1   # U columns: 20 x-features + ones
H = 128
NCH = 16       # j-chunks of 128
NB = 4         # i-blocks
FB = 512       # i-block width
NUM_LAYERS = 2
N_CORES = 8


def _build(ctx, tc, aps, ctot):
    import concourse.bass as bass
    from concourse import mybir
    from concourse.masks import make_identity

    nc = tc.nc
    f32 = mybir.dt.float32
    f16 = mybir.dt.float16
    Alu = mybir.AluOpType
    Act = mybir.ActivationFunctionType

    x_ap, w21_ap, v_ap, ident_ap, s0_ap, sel_ap, p0_ap, g0_ap, out_ap = aps

    consts = ctx.enter_context(tc.tile_pool(name="consts", bufs=1))
    utp = ctx.enter_context(tc.tile_pool(name="utp", bufs=2))        # U0T / YUT rows [UD, N]
    natp = ctx.enter_context(tc.tile_pool(name="natp", bufs=4))      # U_nat f32 [128, 16, UD]
    natp16 = ctx.enter_context(tc.tile_pool(name="natp16", bufs=4))  # U_nat fp16
    ynat = ctx.enter_context(tc.tile_pool(name="ynat", bufs=2))      # Ynat f32 [128, 16, UD]
    xin = ctx.enter_context(tc.tile_pool(name="xin", bufs=4))        # x load tiles
    gpool = ctx.enter_context(tc.tile_pool(name="gpool", bufs=4))    # gbc [128, 512]
    epool = ctx.enter_context(tc.tile_pool(name="epool", bufs=8))    # E tiles [128, 512] f16
    outp = ctx.enter_context(tc.tile_pool(name="outp", bufs=2))      # final hidden [128,16,128]
    small = ctx.enter_context(tc.tile_pool(name="small", bufs=12))
    psA = ctx.enter_context(tc.tile_pool(name="psA", bufs=1, space="PSUM"))  # ubc [128,512]
    psU = ctx.enter_context(tc.tile_pool(name="psU", bufs=4, space="PSUM"))  # YUT [UD,512]
    psT = ctx.enter_context(tc.tile_pool(name="psT", bufs=3, space="PSUM"))  # transposes

    # DMA queue order = first-consumption order: the first PE ops are the
    # L0 selector matmuls (need selmat16 + g0), then the sweep needs un16
    # (xflat) and the E scalars (p0).
    selmat16 = consts.tile([NCH, NCH, 128], f16)
    nc.sync.dma_start(out=selmat16, in_=sel_ap)
    s0_tiles, p0_tiles, g0_tiles, xflats = [], [], [], []
    for s in range(S):
        g0 = small.tile([NCH, 128], f16, tag="g16", name=f"g0_{s}")
        nc.sync.dma_start(out=g0, in_=g0_ap[s])
        g0_tiles.append(g0)
    for s in range(S):
        xflat = xin.tile([128, NCH, Din], f32, name=f"xflat{s}")
        nc.sync.dma_start(out=xflat, in_=x_ap[s].rearrange("(p c) d -> p c d", c=NCH))
        xflats.append(xflat)
        p0 = small.tile([128, NCH, 2], f32, tag="p0", name=f"p0_{s}")
        nc.sync.dma_start(out=p0, in_=p0_ap[s])
        p0_tiles.append(p0)
    for s in range(S):
        s0 = small.tile([128, NCH, 2], f32, tag="s0", name=f"s0_{s}")
        nc.sync.dma_start(out=s0, in_=s0_ap[s])
        s0_tiles.append(s0)
    ident = consts.tile([128, 128], f32)
    nc.sync.dma_start(out=ident, in_=ident_ap)
    ones_r = consts.tile([1, 128], f32)
    nc.vector.memset(ones_r, 1.0)
    w21_sb = consts.tile([UD, 2], f32)
    nc.sync.dma_start(out=w21_sb, in_=w21_ap)
    v_sb = consts.tile([UD, H], f32)
    nc.sync.dma_start(out=v_sb, in_=v_ap)

    def ts(out, in0, s1, s2, op0, op1=None):
        if op1 is None:
            nc.vector.tensor_scalar(out, in0, s1, None, op0)
        else:
            nc.vector.tensor_scalar(out, in0, s1, s2, op0, op1)

    # ------------- input stage: x -> U0 (natural + T), initial scores -------
    u_nat = [None, None]
    u_nat16 = [None, None]
    s_part = [None, None]   # biasless scores [128, 16, 2]
    for s in range(S):
        # node n lives at (partition p, chunk c) with n = 16 p + c —
        # a fixed relabeling the attention sum is invariant to.
        # Initial biasless scores s0 = [x|1] @ w21 are input-only: folded on host.
        xflat = xflats[s]
        un = natp.tile([128, NCH, UD], f32, tag="unat")
        nc.vector.memset(un[:, :, Din:UD], 1.0)
        nc.vector.tensor_copy(un[:, :, 0:Din], xflat)
        un16 = natp16.tile([128, NCH, UD], f16, tag="unat16")
        nc.scalar.copy(un16, un)
        u_nat[s], u_nat16[s], s_part[s] = un, un16, s0_tiles[s]

    # ------------- layers ---------------------------------------------------
    # Emission schedule (PE is in-order per engine): each sample's finalize +
    # next-layer prep is emitted right after its own sweep, so it executes
    # while the OTHER sample's sweep occupies the PE.
    prep = {}
    yuts = {}
    finals = {}

    def emit_prep(s, L):
        un, un16, s0 = u_nat[s], u_nat16[s], s_part[s]
        if L == 0:
            # p, p', g for layer 0 depend only on inputs: host-folded
            p0 = p0_tiles[s]
            gbc = gpool.tile([128, N], f16, tag="gbc")
            for b in range(NB):
                ubc = psA.tile([128, FB], f32, tag="ubc")
                for k in range(4):
                    c = 4 * b + k
                    nc.tensor.matmul(ubc[:, k * 128:(k + 1) * 128],
                                     lhsT=selmat16[:, c, :], rhs=g0_tiles[s],
                                     start=True, stop=True)
                nc.scalar.copy(gbc[:, b * FB:(b + 1) * FB], ubc)
            prep[s] = (p0[:, :, 0], p0[:, :, 1], gbc)
            return
        m1 = small.tile([128, 1], f32, tag="m1")
        nc.vector.tensor_reduce(m1, s0[:, :, 0], axis=mybir.AxisListType.X, op=Alu.max)
        psm = psT.tile([1, 128], f32, tag="tp")
        nc.tensor.matmul(psm, lhsT=m1, rhs=ident, start=True, stop=True)
        m1r = small.tile([1, 128], f32, tag="m1r")
        nc.scalar.copy(m1r, psm)
        mx = small.tile([1, 1], f32, tag="mx")
        nc.vector.tensor_reduce(mx, m1r, axis=mybir.AxisListType.X, op=Alu.max)
        psmb = psT.tile([128, 1], f32, tag="tp")
        nc.tensor.matmul(psmb, lhsT=ones_r, rhs=mx, start=True, stop=True)
        maxbc = small.tile([128, 1], f32, tag="maxbc")
        nc.scalar.copy(maxbc, psmb)
        negmax = small.tile([128, 1], f32, tag="negmax")
        ts(negmax, maxbc, -1.0, None, Alu.mult)
        negmax001 = small.tile([128, 1], f32, tag="negmax001")
        ts(negmax001, maxbc, -0.01, None, Alu.mult)
        p_sb = small.tile([128, NCH], f32, tag="p_sb")
        nc.scalar.activation(p_sb, s0[:, :, 0], Act.Exp, bias=negmax[:, 0:1], scale=1.0)
        pp_sb = small.tile([128, NCH], f32, tag="pp_sb")
        nc.scalar.activation(pp_sb, s0[:, :, 0], Act.Exp, bias=negmax001[:, 0:1], scale=0.01)
        u1 = small.tile([128, NCH], f32, tag="u1")
        ts(u1, s0[:, :, 1], maxbc[:, 0:1], float(ctot), Alu.add, Alu.add)
        u_sb = small.tile([128, NCH], f32, tag="u_sb")
        ts(u_sb, u1, -0.99, 10.5, Alu.mult, Alu.min)
        psuT = psT.tile([NCH, 128], f32, tag="tp")
        nc.tensor.transpose(psuT, u_sb, ident)
        g16 = small.tile([NCH, 128], f16, tag="g16")
        nc.scalar.activation(g16, psuT, Act.Exp)
        gbc = gpool.tile([128, N], f16, tag="gbc")
        for b in range(NB):
            ubc = psA.tile([128, FB], f32, tag="ubc")
            for k in range(4):
                c = 4 * b + k
                nc.tensor.matmul(ubc[:, k * 128:(k + 1) * 128],
                                 lhsT=selmat16[:, c, :], rhs=g16,
                                 start=True, stop=True)
            nc.scalar.copy(gbc[:, b * FB:(b + 1) * FB], ubc)
        prep[s] = (p_sb, pp_sb, gbc)

    def emit_sweep(s):
        p_sb, pp_sb, gbc = prep[s]
        un16 = u_nat16[s]
        yut_sb = utp.tile([UD, N], f32, tag="yut", name=f"yut{s}")
        W2 = 2 * FB
        for bb in range(NB // 2):
            yps0 = psU.tile([UD, FB], f32, tag="yps", name="yps0")
            yps1 = psU.tile([UD, FB], f32, tag="yps", name="yps1")
            etiles = []
            for c in range(NCH):
                e_t = epool.tile([128, W2], f16, tag="e", name=f"e{c}")
                ts(e_t, gbc[:, bb * W2:(bb + 1) * W2], pp_sb[:, c:c + 1],
                   p_sb[:, c:c + 1], Alu.mult, Alu.max)
                etiles.append(e_t)
            for c in range(NCH):
                nc.tensor.matmul(yps0, lhsT=un16[:, c, :], rhs=etiles[c][:, 0:FB],
                                 start=(c == 0), stop=(c == NCH - 1))
                nc.tensor.matmul(yps1, lhsT=un16[:, c, :], rhs=etiles[c][:, FB:W2],
                                 start=(c == 0), stop=(c == NCH - 1))
            nc.scalar.copy(yut_sb[:, bb * W2:bb * W2 + FB], yps0)
            nc.scalar.copy(yut_sb[:, bb * W2 + FB:(bb + 1) * W2], yps1)
        yuts[s] = yut_sb

    def emit_fin(s, L, last):
        un, s0, yut_sb = u_nat[s], s_part[s], yuts[s]
        yn = ynat.tile([128, NCH, UD], f32, tag="ynat")
        for c in range(NCH):
            pst = psT.tile([128, UD], f32, tag="tp")
            nc.tensor.transpose(pst, yut_sb[:, c * 128:(c + 1) * 128],
                                ident[0:UD, 0:UD])
            nc.scalar.copy(yn[:, c, :], pst)
        dsc = small.tile([128, NCH], f32, tag="dsc")
        ts(dsc, yn[:, :, Din], float(2.0 ** (-L)), None, Alu.mult)
        rd = small.tile([128, NCH], f32, tag="rd")
        nc.vector.reciprocal(rd, dsc)
        new_un = natp.tile([128, NCH, UD], f32, tag="unat")
        for c in range(NCH):
            nc.vector.scalar_tensor_tensor(new_un[:, c, :], yn[:, c, :],
                                           rd[:, c:c + 1], un[:, c, :],
                                           Alu.mult, Alu.add)
        if not last:
            new_un16 = natp16.tile([128, NCH, UD], f16, tag="unat16")
            nc.scalar.copy(new_un16, new_un)
            psq = psT.tile([128, 32], f32, tag="tp")
            for c in range(NCH):
                nc.tensor.matmul(psq[:, 2 * c:2 * c + 2],
                                 lhsT=yut_sb[:, c * 128:(c + 1) * 128],
                                 rhs=w21_sb, start=True, stop=True)
            qp = small.tile([128, NCH, 2], f32, tag="qp")
            nc.scalar.copy(qp, psq.rearrange("p (c z) -> p c z", z=2))
            new_s0 = small.tile([128, NCH, 2], f32, tag="s0")
            for c in range(NCH):
                nc.vector.scalar_tensor_tensor(new_s0[:, c, :], qp[:, c, :],
                                               rd[:, c:c + 1], s0[:, c, :],
                                               Alu.mult, Alu.add)
            u_nat[s], u_nat16[s], s_part[s] = new_un, new_un16, new_s0
        else:
            finals[s] = new_un

    # L0: fin+prep(L1) of each sample emitted right after its own sweep
    emit_prep(0, 0)
    emit_prep(1, 0)
    emit_sweep(0)
    emit_fin(0, 0, last=False)
    emit_prep(0, 1)
    emit_sweep(1)
    emit_fin(1, 0, last=False)
    emit_prep(1, 1)
    # L1
    emit_sweep(0)
    emit_fin(0, 1, last=True)
    emit_sweep(1)
    emit_fin(1, 1, last=True)

    # final tail: hidden = U' @ V, samples interleaved, grouped output DMA
    houts = {s: outp.tile([128, NCH, H], f32, tag="hout", name=f"hout{s}")
             for s in range(S)}
    for c in range(NCH):
        for s in range(S):
            psut = psU.tile([UD, 128], f32, tag="yps")
            nc.tensor.transpose(psut, finals[s][:, c, :], ident)
            u2t_c = small.tile([UD, 128], f32, tag="u2t")
            nc.scalar.copy(u2t_c, psut)
            psh = psT.tile([128, H], f32, tag="tp")
            nc.tensor.matmul(psh, lhsT=u2t_c, rhs=v_sb, start=True, stop=True)
            nc.vector.tensor_copy(houts[s][:, c, :], psh)
        if c % 4 == 3:
            for s in range(S):
                nc.sync.dma_start(
                    out=out_ap[s].rearrange("(p c) h -> p c h", c=NCH)[:, c - 3:c + 1, :],
                    in_=houts[s][:, c - 3:c + 1, :])

def _host_prep(inputs):
    x = np.ascontiguousarray(np.asarray(inputs["x"], dtype=np.float32))
    W_in = np.asarray(inputs["W_in"], dtype=np.float32)
    b_in = np.asarray(inputs["b_in"], dtype=np.float32)
    W_t = np.asarray(inputs["W_t"], dtype=np.float32)
    b_t = np.asarray(inputs["b_t"], dtype=np.float32)
    a = np.asarray(inputs["a"], dtype=np.float32)
    a_j, a_i = a[:H, 0], a[H:, 0]
    wj = (W_t @ a_j).astype(np.float32)
    wi = (W_t @ a_i).astype(np.float32)
    V = np.ascontiguousarray(np.concatenate([W_in, b_in[None, :]], axis=0))  # [21, 128]
    w21 = np.ascontiguousarray(np.stack([V @ wj, V @ wi], axis=1))           # [21, 2]
    ctot = float(np.float32(b_t @ a_j) + np.float32(b_t @ a_i))
    B = x.shape[0]
    U0 = np.concatenate([x, np.ones((B, N, 1), np.float32)], axis=2)
    s0 = (U0 @ w21).astype(np.float32).reshape(B, 128, NCH, 2)  # n = 16p + c
    s0 = np.ascontiguousarray(s0)
    sel = np.zeros((NCH, NCH, 128), np.float16)
    for c in range(NCH):
        sel[c, c, :] = 1.0
    s0j, s0i = s0[..., 0], s0[..., 1]
    mx = s0j.max(axis=(1, 2), keepdims=True)
    p0 = np.stack([np.exp(s0j - mx), np.exp(0.01 * (s0j - mx))], axis=3).astype(np.float32)
    u0 = np.minimum(-0.99 * (s0i + mx + np.float32(ctot)), 10.5).astype(np.float32)
    g0 = np.ascontiguousarray(np.exp(u0).astype(np.float16).transpose(0, 2, 1))
    return x, w21, V, ctot, s0, sel, p0, g0


def build_program(ctot):
    import concourse.tile as tile
    from concourse import mybir
    from concourse.bacc import Bacc

    f32 = mybir.dt.float32
    nc = Bacc("TRN2", target_bir_lowering=False, debug=False)
    x_t = nc.dram_tensor("x", [S, N, Din], f32, kind="ExternalInput")
    w21_t = nc.dram_tensor("w21", [UD, 2], f32, kind="ExternalInput")
    v_t = nc.dram_tensor("v", [UD, H], f32, kind="ExternalInput")
    ident_t = nc.dram_tensor("ident", [128, 128], f32, kind="ExternalInput")
    s0_t = nc.dram_tensor("s0in", [S, 128, NCH, 2], f32, kind="ExternalInput")
    sel_t = nc.dram_tensor("sel16", [NCH, NCH, 128], mybir.dt.float16, kind="ExternalInput")
    p0_t = nc.dram_tensor("p0in", [S, 128, NCH, 2], f32, kind="ExternalInput")
    g0_t = nc.dram_tensor("g0in", [S, NCH, 128], mybir.dt.float16, kind="ExternalInput")
    out_t = nc.dram_tensor("out", [S, N, H], f32, kind="ExternalOutput")
    aps = (x_t.ap(), w21_t.ap(), v_t.ap(), ident_t.ap(), s0_t.ap(), sel_t.ap(), p0_t.ap(), g0_t.ap(), out_t.ap())
    with tile.TileContext(nc) as tc, ExitStack() as ctx:
        _build(ctx, tc, aps, ctot)
    nc.compile()
    return nc


def kernel(**inputs) -> np.ndarray:
    from concourse.bass_utils import run_bass_kernel_spmd

    x, w21, V, ctot, s0, sel, p0, g0 = _host_prep(inputs)
    B = x.shape[0]
    nc = build_program(ctot)
    in_maps = []
    for i in range(N_CORES):
        in_maps.append({
            "x": np.ascontiguousarray(x[i * S:(i + 1) * S]),
            "w21": w21,
            "v": V,
            "ident": np.eye(128, dtype=np.float32),
            "s0in": np.ascontiguousarray(s0[i * S:(i + 1) * S]),
            "sel16": sel,
            "p0in": np.ascontiguousarray(p0[i * S:(i + 1) * S]),
            "g0in": np.ascontiguousarray(g0[i * S:(i + 1) * S]),
        })
    res = run_bass_kernel_spmd(nc, in_maps, list(range(N_CORES)))
    out = np.concatenate([res.results[i]["out"] for i in range(N_CORES)], axis=0)
    assert out.shape == (B, N, H)
    return out
